# revision 43
# baseline (speedup 1.0000x reference)
"""Trainium2 Bass kernel for unscaled Luong dot-product attention.

Problem: B=16, Tq=Tk=D=1024, fp32.
    scores = Q @ E^T ; weights = softmax(scores, -1) ; out = weights @ E

Sharding: data-parallel over batch — each of the 8 NeuronCores processes
2 batches end-to-end; no cross-core communication.

Per-core pipeline (per batch, per 128-row q-block):
  1. Round Q/E to float32r up front (DVE/Pool copies) and PE-transpose the
     pre-rounded tiles into [D-part, T-free] layout at the f32r transpose
     rate (1.5 cyc/row vs 2.0 for fp32). HW float32r matmul keeps ~16
     effective mantissa bits, so a single f32r pass lands ~8e-4 rel err —
     far inside the 2e-2 gate — and the 3xTF32 residual split the earlier
     revision used (npass=3) is unnecessary.
  2. bmm1: scores[q,k] accumulated over 8 d-chunks in PSUM (one f32r pass),
     d-chunk-outer / bank-inner order (ldw_min) so each stationary Q chunk
     is loaded once for both 512-wide PSUM halves.
  3. Softmax along the free dim: DVE reduce_max per 512-half as soon as its
     PSUM bank closes (nm2), combined with min -> one ACT exp per half with
     per-partition bias, written directly as f32r, with fused row-sum
     accumulation -> DVE reciprocal folded into the output copy.
  4. PE-transpose the f32r weights block and run bmm2 against E kept in
     natural [k,d] f32r layout (single pass).
Cross-batch software pipeline (e2): E tiles for batch b+1 are DMA'd,
rounded, and PE-transposed in the shadow of batch b's last softmax/bmm2,
double-buffering e_r/etr, so the PE never idles at batch boundaries.
"""

import numpy as np

import concourse.bass as bass
import concourse.tile as tile
from concourse import bacc, mybir
from concourse.masks import make_identity

P = 128
B_PER_CORE = 2
T = 1024  # Tq = Tk
D = 1024
NC_CHUNKS = T // P  # 8 k-chunks / q-blocks
ND_CHUNKS = D // P  # 8 d-chunks
F32 = mybir.dt.float32
F32R = mybir.dt.float32r
BF16 = mybir.dt.bfloat16


def _transpose_block_group(
    nc, trans_psum, ident, src_fn, dst_r, dst_l, n_blocks=4, copy_eng=None, dtype=F32
):
    """Transpose `n_blocks` [128,128] SBUF blocks through one PSUM bank,
    then round the packed result into `dst_r` and (optionally) the
    residual into float32r `dst_l` (3xTF32 split). src_fn(j) -> source AP.
    The PSUM tile + identity use `dtype`: f32r sources transpose at 1.5
    cycles/row, bf16 at 1.0, vs 2.0 for fp32. copy_eng picks the
    PSUM->SBUF copy engine (callers alternate ACT/DVE so neither engine
    serializes the transpose chain); residual is DVE-only."""
    tp = trans_psum.tile([P, n_blocks * P], dtype)
    for j in range(n_blocks):
        nc.tensor.transpose(tp[:, j * P : (j + 1) * P], src_fn(j), ident)
    if copy_eng is None:
        copy_eng = nc.scalar
    if copy_eng is nc.scalar:
        nc.scalar.copy(dst_r, tp[:])
    else:
        nc.vector.tensor_copy(dst_r, tp[:])
    if dst_l is not None:
        nc.vector.tensor_tensor(dst_l, tp[:], dst_r, mybir.AluOpType.subtract)


def build_nc(
    reps: int = 1,
    npass: int = 1,
    dma_only: bool = False,
    e_hoist: bool = False,
    no_softmax: bool = False,
    e_only: bool = False,
    e_sync_dma: bool = False,
    ldw_min: bool = True,
    deep_bufs: bool = False,
    w_split: bool = True,
    qw_half: bool = False,
    ctx2: bool = False,
    ldw2: bool = False,
    tr_f32r: bool = True,
    w_bf16: bool = False,
    bmm2_kc: bool = False,
    e2: bool = True,
    nm2: bool = True,
    et_f32: bool = False,
):
    nc = bacc.Bacc("TRN2", target_bir_lowering=False, debug=False)
    q_dram = nc.dram_tensor("q", [B_PER_CORE, T, D], F32, kind="ExternalInput").ap()
    e_dram = nc.dram_tensor("e", [B_PER_CORE, T, D], F32, kind="ExternalInput").ap()
    o_dram = nc.dram_tensor("o", [B_PER_CORE, T, D], F32, kind="ExternalOutput").ap()

    with tile.TileContext(nc) as tc:
        with (
            tc.tile_pool(name="const", bufs=1) as const_pool,
            tc.tile_pool(name="e_nat", bufs=6) as e_nat_pool,
            tc.tile_pool(name="e_r", bufs=2 if e2 else 1) as e_r_pool,
            tc.tile_pool(name="e_rs", bufs=3) as e_rs_pool,
            tc.tile_pool(name="e_rb", bufs=2 if e2 else 1) as e_rb_pool,
            tc.tile_pool(name="etr", bufs=2 if e2 else 1) as etr_pool,
            tc.tile_pool(name="etl", bufs=1) as etl_pool,
            tc.tile_pool(name="qstage", bufs=3 if deep_bufs else 2) as q_pool,
            tc.tile_pool(name="qr", bufs=3 if deep_bufs else 2) as qr_pool,
            tc.tile_pool(name="qt", bufs=3 if deep_bufs else 2) as qt_pool,
            tc.tile_pool(name="w", bufs=3 if deep_bufs else 2) as w_pool,
            tc.tile_pool(name="wt", bufs=3 if deep_bufs else 2) as wt_pool,
            tc.tile_pool(name="ctx", bufs=3 if deep_bufs else 2) as ctx_pool,
            tc.tile_pool(name="stat", bufs=4) as stat_pool,
            tc.tile_pool(name="sc_ps", bufs=2, space="PSUM") as sc_psum,
            tc.tile_pool(
                name="ctx_ps", bufs=2 if ctx2 else 1, space="PSUM"
            ) as ctx_psum,
            tc.tile_pool(
                name="tr_ps", bufs=2 if (ctx2 or bmm2_kc) else 3, space="PSUM"
            ) as trans_psum,
        ):
            ident = const_pool.tile([P, P], F32)
            make_identity(nc, ident[:])
            ident_r = None
            if tr_f32r:
                assert npass == 1, "tr_f32r transposes pre-rounded data; no residual"
                ident_r = const_pool.tile([P, P], F32R)
                nc.vector.tensor_copy(ident_r[:], ident[:])
            ident_b = None
            if w_bf16:
                ident_b = const_pool.tile([P, P], BF16)
                nc.vector.tensor_copy(ident_b[:], ident[:])

            ngr = 2 if qw_half else 1  # half-tiles per transposed operand
            nper = ND_CHUNKS // ngr  # d/k-chunks per half-tile

            def qsel(tiles, c):
                return tiles[c // nper][:, c % nper, :]

            def emit_e_setup(b):
                # ---- E setup, pipelined per 128-row chunk ----
                # Small staging tiles (bufs=3) let chunk k+1's DMA overlap
                # chunk k's transposes, and let the next batch's E DMAs start
                # during this batch's compute. gpsimd (SWDGE) queue keeps them
                # out of the sync queue behind the output DMAs.
                # bmm2 weights are bf16 when w_bf16 and walrus refuses mixed
                # 32/16-bit matmul inputs, so bmm2 reads E as bf16 (e_rb) and
                # the persistent f32r copy is unnecessary — f32r rounding for
                # the etr transposes goes through a small staging tile instead
                e_r = (
                    None
                    if w_bf16
                    else e_r_pool.tile([P, NC_CHUNKS, D], F32R, name="e_r")
                )
                e_rb = (
                    e_rb_pool.tile([P, NC_CHUNKS, D], BF16, name="e_rb")
                    if w_bf16
                    else None
                )
                etr = etr_pool.tile([P, ND_CHUNKS, T], F32R, name="etr")
                etl = (
                    etl_pool.tile([P, ND_CHUNKS, T], F32R, tag="etl", name="etl")
                    if npass >= 3
                    else None
                )
                dma_eng = nc.sync if e_sync_dma else nc.gpsimd
                for kc in range(NC_CHUNKS):
                    e_stage = e_nat_pool.tile([P, D], F32, name="e_stage")
                    dma_eng.dma_start(
                        e_stage[:], e_dram[b, kc * P : (kc + 1) * P, :]
                    )
                    if e_rb is not None:
                        nc.gpsimd.tensor_copy(e_rb[:, kc, :], e_stage[:])
                    if e_r is not None:
                        nc.vector.tensor_copy(e_r[:, kc, :], e_stage[:])
                    # transpose the chunk's 8 [128,128] blocks -> column kc of
                    # each etr[:, dc, :]; pack 4 d-blocks per PSUM bank.
                    # tr_f32r: source pre-rounded f32r data so the PE
                    # transpose runs at 1.5 cyc/row instead of fp32's 2.0
                    if tr_f32r and e_r is not None:
                        tsrc, et_dt = e_r[:, kc, :], F32R
                    elif tr_f32r and not et_f32:
                        e_rs = e_rs_pool.tile([P, D], F32R, name="e_rs")
                        nc.vector.tensor_copy(e_rs[:], e_stage[:])
                        tsrc, et_dt = e_rs[:], F32R
                    else:
                        tsrc, et_dt = e_stage[:], F32
                    for g in range(ND_CHUNKS // 4):
                        _transpose_block_group(
                            nc,
                            trans_psum,
                            ident_r[:] if et_dt is F32R else ident[:],
                            lambda j, tsrc=tsrc, g=g: tsrc[
                                :, (g * 4 + j) * P : (g * 4 + j + 1) * P
                            ],
                            etr[:, g * 4 : (g + 1) * 4, kc * P : (kc + 1) * P],
                            etl[:, g * 4 : (g + 1) * 4, kc * P : (kc + 1) * P]
                            if etl is not None
                            else None,
                            copy_eng=nc.scalar if (kc * 2 + g) % 2 == 0 else nc.vector,
                            dtype=et_dt,
                        )
                return e_r, e_rb, etr, etl

            batches = [b for _ in range(reps) for b in range(B_PER_CORE)]
            e_cache = None
            e_next = None
            for bi, b in enumerate(batches):
                if dma_only:
                    for kc in range(NC_CHUNKS):
                        e_stage = e_nat_pool.tile([P, D], F32, name="e_stage")
                        nc.gpsimd.dma_start(
                            e_stage[:], e_dram[b, kc * P : (kc + 1) * P, :]
                        )
                    for qb in range(NC_CHUNKS):
                        qstage = q_pool.tile([P, D], F32, name="qstage")
                        nc.sync.dma_start(
                            qstage[:], q_dram[b, qb * P : (qb + 1) * P, :]
                        )
                        ctx_sb = ctx_pool.tile([P, D], F32, name="ctx_sb")
                        nc.vector.tensor_copy(ctx_sb[:], qstage[:])
                        nc.sync.dma_start(
                            o_dram[b, qb * P : (qb + 1) * P, :], ctx_sb[:]
                        )
                    continue
                if e_next is not None:
                    e_cache = e_next
                    e_next = None
                elif e_cache is None or not e_hoist:
                    e_cache = emit_e_setup(b)
                e_r, e_rb, etr, etl = e_cache
                if e_only:
                    # ablation: skip all q-block work; touch etr so it isn't dead
                    ctx_sb = ctx_pool.tile([P, D], F32, name="ctx_sb")
                    nc.vector.tensor_copy(ctx_sb[:], etr[:, 0, :])
                    nc.sync.dma_start(o_dram[b, 0:P, :], ctx_sb[:])
                    continue

                def emit_front(qb, b=b, etr=etr, etl=etl):
                    """Stage Q block qb, transpose+split it, run bmm1.
                    Returns the scores PSUM tile."""
                    qstage = q_pool.tile([P, D], F32, name="qstage")
                    nc.sync.dma_start(qstage[:], q_dram[b, qb * P : (qb + 1) * P, :])
                    if tr_f32r:
                        # Pre-round on the otherwise-idle Pool engine so the
                        # PE transposes run at the f32r rate.
                        q_r = qr_pool.tile([P, D], F32R, name="q_r")
                        nc.gpsimd.tensor_copy(q_r[:], qstage[:])
                        q_tsrc = q_r
                    else:
                        q_tsrc = qstage
                    qtr_t = [
                        qt_pool.tile(
                            [P, ND_CHUNKS // ngr, P], F32R,
                            tag=f"qtr{h}", name=f"qtr{h}",
                        )
                        for h in range(ngr)
                    ]
                    qtl_t = [
                        qt_pool.tile(
                            [P, ND_CHUNKS // ngr, P], F32R,
                            tag=f"qtl{h}", name=f"qtl{h}",
                        )
                        for h in range(ngr)
                    ] if npass >= 2 else None
                    for g in range(ND_CHUNKS // 4):
                        o = (g * 4) % nper
                        _transpose_block_group(
                            nc,
                            trans_psum,
                            ident_r[:] if tr_f32r else ident[:],
                            lambda j, g=g: q_tsrc[
                                :, (g * 4 + j) * P : (g * 4 + j + 1) * P
                            ],
                            qtr_t[(g * 4) // nper][:, o : o + 4, :],
                            qtl_t[(g * 4) // nper][:, o : o + 4, :]
                            if qtl_t is not None
                            else None,
                            copy_eng=nc.scalar if g % 2 == 0 else nc.vector,
                            dtype=F32R if tr_f32r else F32,
                        )

                    # bmm1: bank-contiguous bursts (kh outer), npass x 8 k-chunks
                    sc_ps = sc_psum.tile([P, T], F32, name="sc_ps")
                    pairs = [(qtr_t, etr), (qtl_t, etr), (qtr_t, etl)][:npass]
                    n_acc = len(pairs) * ND_CHUNKS
                    if ldw_min:
                        # group MMs by stationary operand: 1 LDW per 4 MMs
                        groups = {}
                        for lhs, rhs in pairs:
                            groups.setdefault(id(lhs), (lhs, []))[1].append(rhs)
                        seq = []  # (lhs, rhs, kh)
                        for dc in range(ND_CHUNKS):
                            for lhs, rhss in groups.values():
                                for rhs in rhss:
                                    for kh in range(2):
                                        seq.append((lhs, rhs, dc, kh))
                        started = set()
                        for i, (lhs, rhs, dc, kh) in enumerate(seq):
                            nc.tensor.matmul(
                                sc_ps[:, kh * 512 : (kh + 1) * 512],
                                qsel(lhs, dc),
                                rhs[:, dc, kh * 512 : (kh + 1) * 512],
                                start=(kh not in started),
                                stop=(i >= len(seq) - 2),
                            )
                            started.add(kh)
                    else:
                        for kh in range(2):
                            if ldw2 and npass == 3:
                                # group the two qtr-consuming passes per
                                # d-chunk: one weight load serves two MMs,
                                # same PSUM bank throughout the half
                                seq = [
                                    (lhs, rhs, dc)
                                    for dc in range(ND_CHUNKS)
                                    for lhs, rhs in ((qtr_t, etr), (qtr_t, etl))
                                ] + [
                                    (qtl_t, etr, dc) for dc in range(ND_CHUNKS)
                                ]
                            else:
                                seq = [
                                    (lhs, rhs, dc)
                                    for lhs, rhs in pairs
                                    for dc in range(ND_CHUNKS)
                                ]
                            for i, (lhs, rhs, dc) in enumerate(seq):
                                nc.tensor.matmul(
                                    sc_ps[:, kh * 512 : (kh + 1) * 512],
                                    qsel(lhs, dc),
                                    rhs[:, dc, kh * 512 : (kh + 1) * 512],
                                    start=(i == 0),
                                    stop=(i == len(seq) - 1),
                                )
                    return sc_ps

                def emit_back(qb, sc_ps, b=b, e_r=e_r, e_rb=e_rb):
                    """Softmax block qb's scores, transpose W, bmm2, store."""
                    recip = stat_pool.tile([P, 1], F32, tag="recip", name="recip")
                    # tr_f32r: ACT writes the exp output pre-rounded to f32r
                    # (free on the output path) so the W transposes also run
                    # at 1.5 cyc/row
                    wdt = BF16 if w_bf16 else (F32R if tr_f32r else F32)
                    if w_split:
                        # two half-tiles: each half's W transposes start as
                        # soon as its own exp half is done
                        w_halves = [
                            w_pool.tile([P, T // 2], wdt, tag=f"w{h}", name=f"w{h}")
                            for h in range(2)
                        ]
                    else:
                        w_sb = w_pool.tile([P, T], wdt, name="w_sb")
                        w_halves = [w_sb[:, 0:512], w_sb[:, 512:1024]]
                    if no_softmax:
                        nc.scalar.copy(w_halves[0][:], sc_ps[:, 0:512])
                        nc.scalar.copy(w_halves[1][:], sc_ps[:, 512:1024])
                        nc.vector.memset(recip[:], 1.0)
                    else:
                        negmax = stat_pool.tile(
                            [P, 1], F32, tag="negmax", name="negmax"
                        )
                        if nm2:
                            # halve the row-max latency: the kh=0 PSUM bank is
                            # complete at bmm1's midpoint, so its reduce runs
                            # under the second half of bmm1; combine with min
                            nmh = [
                                stat_pool.tile(
                                    [P, 1], F32, tag=f"nmh{h}", name=f"nmh{h}"
                                )
                                for h in range(2)
                            ]
                            for h in range(2):
                                nc.vector.tensor_reduce(
                                    out=nmh[h][:],
                                    in_=sc_ps[:, h * 512 : (h + 1) * 512],
                                    op=mybir.AluOpType.max,
                                    axis=mybir.AxisListType.X,
                                    negate=True,
                                )
                            nc.vector.tensor_tensor(
                                negmax[:], nmh[0][:], nmh[1][:],
                                mybir.AluOpType.min,
                            )
                        else:
                            nc.vector.tensor_reduce(
                                out=negmax[:],
                                in_=sc_ps[:],
                                op=mybir.AluOpType.max,
                                axis=mybir.AxisListType.X,
                                negate=True,
                            )
                        if w_split:
                            ssums = [
                                stat_pool.tile(
                                    [P, 1], F32, tag=f"ssum{h}", name=f"ssum{h}"
                                )
                                for h in range(2)
                            ]
                            for h in range(2):
                                nc.scalar.activation(
                                    w_halves[h][:],
                                    sc_ps[:, h * 512 : (h + 1) * 512],
                                    mybir.ActivationFunctionType.Exp,
                                    bias=negmax[:],
                                    accum_out=ssums[h][:],
                                )
                            ssum = stat_pool.tile(
                                [P, 1], F32, tag="ssum", name="ssum"
                            )
                            nc.vector.tensor_tensor(
                                ssum[:], ssums[0][:], ssums[1][:],
                                mybir.AluOpType.add,
                            )
                        else:
                            ssum = stat_pool.tile(
                                [P, 1], F32, tag="ssum", name="ssum"
                            )
                            nc.scalar.activation(
                                w_sb[:],
                                sc_ps[:],
                                mybir.ActivationFunctionType.Exp,
                                bias=negmax[:],
                                accum_out=ssum[:],
                            )
                        nc.vector.reciprocal(recip[:], ssum[:])

                    wt_t = [
                        wt_pool.tile(
                            [P, NC_CHUNKS // ngr, P], wdt if w_bf16 else F32R,
                            tag=f"wt{h}", name=f"wt{h}",
                        )
                        for h in range(ngr)
                    ]
                    for g in range(NC_CHUNKS // 4):
                        wo = (g * 4) % nper
                        _transpose_block_group(
                            nc,
                            trans_psum,
                            ident_b[:]
                            if w_bf16
                            else (ident_r[:] if tr_f32r else ident[:]),
                            lambda j, g=g: w_halves[g][
                                :, (j * P) : (j + 1) * P
                            ],
                            wt_t[(g * 4) // nper][:, wo : wo + 4, :],
                            None,
                            copy_eng=nc.scalar if g % 2 == 0 else nc.vector,
                            dtype=wdt,
                        )

                    # bmm2: ctx[q,d] = WT.T @ E
                    e2src = e_rb if w_bf16 else e_r
                    ctx_sb = ctx_pool.tile([P, D], F32, name="ctx_sb")
                    if bmm2_kc:
                        # kc-outer: both d-halves consume one weight load of
                        # wt[kc]; needs a 2-bank ctx PSUM tile
                        ctx_ps = ctx_psum.tile([P, T], F32, name="ctx_ps")
                        for kc in range(NC_CHUNKS):
                            for dh in range(2):
                                nc.tensor.matmul(
                                    ctx_ps[:, dh * 512 : (dh + 1) * 512],
                                    qsel(wt_t, kc),
                                    e2src[:, kc, dh * 512 : (dh + 1) * 512],
                                    start=(kc == 0),
                                    stop=(kc == NC_CHUNKS - 1),
                                )
                        for dh in range(2):
                            nc.vector.tensor_scalar_mul(
                                ctx_sb[:, dh * 512 : (dh + 1) * 512],
                                ctx_ps[:, dh * 512 : (dh + 1) * 512],
                                recip[:],
                            )
                    else:
                        # one PSUM bank per d-half
                        for dh in range(2):
                            ctx_ps = ctx_psum.tile([P, 512], F32, name="ctx_ps")
                            for kc in range(NC_CHUNKS):
                                nc.tensor.matmul(
                                    ctx_ps[:],
                                    qsel(wt_t, kc),
                                    e2src[:, kc, dh * 512 : (dh + 1) * 512],
                                    start=(kc == 0),
                                    stop=(kc == NC_CHUNKS - 1),
                                )
                            nc.vector.tensor_scalar_mul(
                                ctx_sb[:, dh * 512 : (dh + 1) * 512], ctx_ps[:], recip[:]
                            )
                    nc.sync.dma_start(o_dram[b, qb * P : (qb + 1) * P, :], ctx_sb[:])

                # software pipeline: next block's bmm1 hides this block's softmax
                pend = emit_front(0)
                for qb in range(NC_CHUNKS):
                    nxt = emit_front(qb + 1) if qb + 1 < NC_CHUNKS else None
                    if (
                        e2
                        and qb == NC_CHUNKS - 1
                        and bi + 1 < len(batches)
                        and not e_hoist
                    ):
                        # prefetch next batch's E pipeline so the PE fills the
                        # last block's softmax/bmm2 shadow with E transposes
                        e_next = emit_e_setup(batches[bi + 1])
                    emit_back(qb, pend)
                    pend = nxt

    nc.compile()
    return nc


_NC_CACHE = None


def _get_nc():
    global _NC_CACHE
    if _NC_CACHE is None:
        _NC_CACHE = build_nc()
    return _NC_CACHE


def kernel(decoder_hidden: np.ndarray, encoder_outputs: np.ndarray) -> np.ndarray:
    import os

    # The axon client here has no NTFF profiling hook; make sure a stray
    # BASS_TRACE in the environment can't push run_bass_kernel_spmd onto
    # the tracing path.
    os.environ["BASS_NEVER_TRACE"] = "1"
    from concourse import bass_utils

    dh = np.ascontiguousarray(np.asarray(decoder_hidden, dtype=np.float32))
    eo = np.ascontiguousarray(np.asarray(encoder_outputs, dtype=np.float32))
    assert dh.shape == (16, T, D) and eo.shape == (16, T, D)

    nc = _get_nc()
    in_maps = [
        {
            "q": dh[i * B_PER_CORE : (i + 1) * B_PER_CORE],
            "e": eo[i * B_PER_CORE : (i + 1) * B_PER_CORE],
        }
        for i in range(8)
    ]
    res = bass_utils.run_bass_kernel_spmd(nc, in_maps, core_ids=list(range(8)))
    return np.concatenate([r["o"] for r in res.results], axis=0)



# revision 46
# speedup vs baseline: 1.2164x; 1.2164x over previous
"""Trainium2 Bass kernel for unscaled Luong dot-product attention.

Problem: B=16, Tq=Tk=D=1024, fp32.
    scores = Q @ E^T ; weights = softmax(scores, -1) ; out = weights @ E

Sharding: data-parallel over batch — each of the 8 NeuronCores processes
2 batches end-to-end; no cross-core communication.

Per-core pipeline (per batch, per 128-row q-block):
  1. Round Q/E to float32r up front (DVE/Pool copies) and PE-transpose the
     pre-rounded tiles into [D-part, T-free] layout at the f32r transpose
     rate (1.5 cyc/row vs 2.0 for fp32). HW float32r matmul keeps ~16
     effective mantissa bits, so a single f32r pass lands ~8e-4 rel err —
     far inside the 2e-2 gate — and the 3xTF32 residual split the earlier
     revision used (npass=3) is unnecessary.
  2. bmm1: scores[q,k] accumulated over 8 d-chunks in PSUM (one f32r pass),
     d-chunk-outer / bank-inner order (ldw_min) so each stationary Q chunk
     is loaded once for both 512-wide PSUM halves.
  3. Softmax along the free dim: DVE reduce_max per 512-half as soon as its
     PSUM bank closes (nm2), combined with min -> one ACT exp per half with
     per-partition bias, written directly as f32r, with fused row-sum
     accumulation -> DVE reciprocal folded into the output copy.
  4. PE-transpose the f32r weights block and run bmm2 against E kept in
     natural [k,d] f32r layout (single pass).
Cross-batch software pipeline (e2): E tiles for batch b+1 are DMA'd,
rounded, and PE-transposed in the shadow of batch b's last softmax/bmm2,
double-buffering e_r/etr, so the PE never idles at batch boundaries.
"""

import numpy as np

import concourse.bass as bass
import concourse.tile as tile
from concourse import bacc, mybir
from concourse.masks import make_identity

P = 128
B_PER_CORE = 2
T = 1024  # Tq = Tk
D = 1024
NC_CHUNKS = T // P  # 8 k-chunks / q-blocks
ND_CHUNKS = D // P  # 8 d-chunks
F32 = mybir.dt.float32
F32R = mybir.dt.float32r
BF16 = mybir.dt.bfloat16


def _transpose_block_group(
    nc, trans_psum, ident, src_fn, dst_r, dst_l, n_blocks=4, copy_eng=None, dtype=F32
):
    """Transpose `n_blocks` [128,128] SBUF blocks through one PSUM bank,
    then round the packed result into `dst_r` and (optionally) the
    residual into float32r `dst_l` (3xTF32 split). src_fn(j) -> source AP.
    The PSUM tile + identity use `dtype`: f32r sources transpose at 1.5
    cycles/row, bf16 at 1.0, vs 2.0 for fp32. copy_eng picks the
    PSUM->SBUF copy engine (callers alternate ACT/DVE so neither engine
    serializes the transpose chain); residual is DVE-only."""
    tp = trans_psum.tile([P, n_blocks * P], dtype)
    for j in range(n_blocks):
        nc.tensor.transpose(tp[:, j * P : (j + 1) * P], src_fn(j), ident)
    if copy_eng is None:
        copy_eng = nc.scalar
    if copy_eng is nc.scalar:
        nc.scalar.copy(dst_r, tp[:])
    elif copy_eng is nc.gpsimd:
        nc.gpsimd.tensor_copy(dst_r, tp[:])
    else:
        nc.vector.tensor_copy(dst_r, tp[:])
    if dst_l is not None:
        nc.vector.tensor_tensor(dst_l, tp[:], dst_r, mybir.AluOpType.subtract)


def build_nc(
    reps: int = 1,
    npass: int = 1,
    dma_only: bool = False,
    e_hoist: bool = False,
    no_softmax: bool = False,
    e_only: bool = False,
    e_sync_dma: bool = False,
    ldw_min: bool = True,
    deep_bufs: bool = False,
    w_split: bool = True,
    qw_half: bool = False,
    ctx2: bool = False,
    ldw2: bool = False,
    tr_f32r: bool = True,
    w_bf16: bool = False,
    bmm2_kc: bool = False,
    e2: bool = True,
    nm2: bool = True,
    et_f32: bool = False,
    pool3: bool = False,
    ctx_act: bool = False,
):
    nc = bacc.Bacc("TRN2", target_bir_lowering=False, debug=False)
    q_dram = nc.dram_tensor("q", [B_PER_CORE, T, D], F32, kind="ExternalInput").ap()
    e_dram = nc.dram_tensor("e", [B_PER_CORE, T, D], F32, kind="ExternalInput").ap()
    o_dram = nc.dram_tensor("o", [B_PER_CORE, T, D], F32, kind="ExternalOutput").ap()

    with tile.TileContext(nc) as tc:
        with (
            tc.tile_pool(name="const", bufs=1) as const_pool,
            tc.tile_pool(name="e_nat", bufs=6) as e_nat_pool,
            tc.tile_pool(name="e_r", bufs=2 if e2 else 1) as e_r_pool,
            tc.tile_pool(name="e_rs", bufs=3) as e_rs_pool,
            tc.tile_pool(name="e_rb", bufs=2 if e2 else 1) as e_rb_pool,
            tc.tile_pool(name="etr", bufs=2 if e2 else 1) as etr_pool,
            tc.tile_pool(name="etl", bufs=1) as etl_pool,
            tc.tile_pool(name="qstage", bufs=3 if deep_bufs else 2) as q_pool,
            tc.tile_pool(name="qr", bufs=3 if deep_bufs else 2) as qr_pool,
            tc.tile_pool(name="qt", bufs=3 if deep_bufs else 2) as qt_pool,
            tc.tile_pool(name="w", bufs=3 if deep_bufs else 2) as w_pool,
            tc.tile_pool(name="wt", bufs=3 if deep_bufs else 2) as wt_pool,
            tc.tile_pool(name="ctx", bufs=3 if deep_bufs else 2) as ctx_pool,
            tc.tile_pool(name="stat", bufs=4) as stat_pool,
            tc.tile_pool(name="sc_ps", bufs=2, space="PSUM") as sc_psum,
            tc.tile_pool(
                name="ctx_ps", bufs=2 if ctx2 else 1, space="PSUM"
            ) as ctx_psum,
            tc.tile_pool(
                name="tr_ps", bufs=2 if (ctx2 or bmm2_kc) else 3, space="PSUM"
            ) as trans_psum,
        ):
            ident = const_pool.tile([P, P], F32)
            make_identity(nc, ident[:])
            ident_r = None
            if tr_f32r:
                assert npass == 1, "tr_f32r transposes pre-rounded data; no residual"
                ident_r = const_pool.tile([P, P], F32R)
                nc.vector.tensor_copy(ident_r[:], ident[:])
            ident_b = None
            if w_bf16:
                ident_b = const_pool.tile([P, P], BF16)
                nc.vector.tensor_copy(ident_b[:], ident[:])

            ngr = 2 if qw_half else 1  # half-tiles per transposed operand
            nper = ND_CHUNKS // ngr  # d/k-chunks per half-tile

            def qsel(tiles, c):
                return tiles[c // nper][:, c % nper, :]

            cp_engs = (
                (nc.scalar, nc.vector, nc.gpsimd)
                if pool3
                else (nc.scalar, nc.vector)
            )

            def emit_e_setup(b):
                # ---- E setup, pipelined per 128-row chunk ----
                # Small staging tiles (bufs=3) let chunk k+1's DMA overlap
                # chunk k's transposes, and let the next batch's E DMAs start
                # during this batch's compute. gpsimd (SWDGE) queue keeps them
                # out of the sync queue behind the output DMAs.
                # bmm2 weights are bf16 when w_bf16 and walrus refuses mixed
                # 32/16-bit matmul inputs, so bmm2 reads E as bf16 (e_rb) and
                # the persistent f32r copy is unnecessary — f32r rounding for
                # the etr transposes goes through a small staging tile instead
                e_r = (
                    None
                    if w_bf16
                    else e_r_pool.tile([P, NC_CHUNKS, D], F32R, name="e_r")
                )
                e_rb = (
                    e_rb_pool.tile([P, NC_CHUNKS, D], BF16, name="e_rb")
                    if w_bf16
                    else None
                )
                etr = etr_pool.tile([P, ND_CHUNKS, T], F32R, name="etr")
                etl = (
                    etl_pool.tile([P, ND_CHUNKS, T], F32R, tag="etl", name="etl")
                    if npass >= 3
                    else None
                )
                dma_eng = nc.sync if e_sync_dma else nc.gpsimd
                for kc in range(NC_CHUNKS):
                    e_stage = e_nat_pool.tile([P, D], F32, name="e_stage")
                    dma_eng.dma_start(
                        e_stage[:], e_dram[b, kc * P : (kc + 1) * P, :]
                    )
                    if e_rb is not None:
                        nc.gpsimd.tensor_copy(e_rb[:, kc, :], e_stage[:])
                    if e_r is not None:
                        nc.vector.tensor_copy(e_r[:, kc, :], e_stage[:])
                    # transpose the chunk's 8 [128,128] blocks -> column kc of
                    # each etr[:, dc, :]; pack 4 d-blocks per PSUM bank.
                    # tr_f32r: source pre-rounded f32r data so the PE
                    # transpose runs at 1.5 cyc/row instead of fp32's 2.0
                    if tr_f32r and e_r is not None:
                        tsrc, et_dt = e_r[:, kc, :], F32R
                    elif tr_f32r and not et_f32:
                        e_rs = e_rs_pool.tile([P, D], F32R, name="e_rs")
                        nc.vector.tensor_copy(e_rs[:], e_stage[:])
                        tsrc, et_dt = e_rs[:], F32R
                    else:
                        tsrc, et_dt = e_stage[:], F32
                    for g in range(ND_CHUNKS // 4):
                        _transpose_block_group(
                            nc,
                            trans_psum,
                            ident_r[:] if et_dt is F32R else ident[:],
                            lambda j, tsrc=tsrc, g=g: tsrc[
                                :, (g * 4 + j) * P : (g * 4 + j + 1) * P
                            ],
                            etr[:, g * 4 : (g + 1) * 4, kc * P : (kc + 1) * P],
                            etl[:, g * 4 : (g + 1) * 4, kc * P : (kc + 1) * P]
                            if etl is not None
                            else None,
                            copy_eng=cp_engs[(kc * 2 + g) % len(cp_engs)],
                            dtype=et_dt,
                        )
                return e_r, e_rb, etr, etl

            batches = [b for _ in range(reps) for b in range(B_PER_CORE)]
            e_cache = None
            e_next = None
            for bi, b in enumerate(batches):
                if dma_only:
                    for kc in range(NC_CHUNKS):
                        e_stage = e_nat_pool.tile([P, D], F32, name="e_stage")
                        nc.gpsimd.dma_start(
                            e_stage[:], e_dram[b, kc * P : (kc + 1) * P, :]
                        )
                    for qb in range(NC_CHUNKS):
                        qstage = q_pool.tile([P, D], F32, name="qstage")
                        nc.sync.dma_start(
                            qstage[:], q_dram[b, qb * P : (qb + 1) * P, :]
                        )
                        ctx_sb = ctx_pool.tile([P, D], F32, name="ctx_sb")
                        nc.vector.tensor_copy(ctx_sb[:], qstage[:])
                        nc.sync.dma_start(
                            o_dram[b, qb * P : (qb + 1) * P, :], ctx_sb[:]
                        )
                    continue
                if e_next is not None:
                    e_cache = e_next
                    e_next = None
                elif e_cache is None or not e_hoist:
                    e_cache = emit_e_setup(b)
                e_r, e_rb, etr, etl = e_cache
                if e_only:
                    # ablation: skip all q-block work; touch etr so it isn't dead
                    ctx_sb = ctx_pool.tile([P, D], F32, name="ctx_sb")
                    nc.vector.tensor_copy(ctx_sb[:], etr[:, 0, :])
                    nc.sync.dma_start(o_dram[b, 0:P, :], ctx_sb[:])
                    continue

                def emit_front(qb, b=b, etr=etr, etl=etl):
                    """Stage Q block qb, transpose+split it, run bmm1.
                    Returns the scores PSUM tile."""
                    qstage = q_pool.tile([P, D], F32, name="qstage")
                    nc.sync.dma_start(qstage[:], q_dram[b, qb * P : (qb + 1) * P, :])
                    if tr_f32r:
                        # Pre-round on the otherwise-idle Pool engine so the
                        # PE transposes run at the f32r rate.
                        q_r = qr_pool.tile([P, D], F32R, name="q_r")
                        nc.gpsimd.tensor_copy(q_r[:], qstage[:])
                        q_tsrc = q_r
                    else:
                        q_tsrc = qstage
                    qtr_t = [
                        qt_pool.tile(
                            [P, ND_CHUNKS // ngr, P], F32R,
                            tag=f"qtr{h}", name=f"qtr{h}",
                        )
                        for h in range(ngr)
                    ]
                    qtl_t = [
                        qt_pool.tile(
                            [P, ND_CHUNKS // ngr, P], F32R,
                            tag=f"qtl{h}", name=f"qtl{h}",
                        )
                        for h in range(ngr)
                    ] if npass >= 2 else None
                    for g in range(ND_CHUNKS // 4):
                        o = (g * 4) % nper
                        _transpose_block_group(
                            nc,
                            trans_psum,
                            ident_r[:] if tr_f32r else ident[:],
                            lambda j, g=g: q_tsrc[
                                :, (g * 4 + j) * P : (g * 4 + j + 1) * P
                            ],
                            qtr_t[(g * 4) // nper][:, o : o + 4, :],
                            qtl_t[(g * 4) // nper][:, o : o + 4, :]
                            if qtl_t is not None
                            else None,
                            copy_eng=cp_engs[(qb * 2 + g) % len(cp_engs)],
                            dtype=F32R if tr_f32r else F32,
                        )

                    # bmm1: bank-contiguous bursts (kh outer), npass x 8 k-chunks
                    sc_ps = sc_psum.tile([P, T], F32, name="sc_ps")
                    pairs = [(qtr_t, etr), (qtl_t, etr), (qtr_t, etl)][:npass]
                    n_acc = len(pairs) * ND_CHUNKS
                    if ldw_min:
                        # group MMs by stationary operand: 1 LDW per 4 MMs
                        groups = {}
                        for lhs, rhs in pairs:
                            groups.setdefault(id(lhs), (lhs, []))[1].append(rhs)
                        seq = []  # (lhs, rhs, kh)
                        for dc in range(ND_CHUNKS):
                            for lhs, rhss in groups.values():
                                for rhs in rhss:
                                    for kh in range(2):
                                        seq.append((lhs, rhs, dc, kh))
                        started = set()
                        for i, (lhs, rhs, dc, kh) in enumerate(seq):
                            nc.tensor.matmul(
                                sc_ps[:, kh * 512 : (kh + 1) * 512],
                                qsel(lhs, dc),
                                rhs[:, dc, kh * 512 : (kh + 1) * 512],
                                start=(kh not in started),
                                stop=(i >= len(seq) - 2),
                            )
                            started.add(kh)
                    else:
                        for kh in range(2):
                            if ldw2 and npass == 3:
                                # group the two qtr-consuming passes per
                                # d-chunk: one weight load serves two MMs,
                                # same PSUM bank throughout the half
                                seq = [
                                    (lhs, rhs, dc)
                                    for dc in range(ND_CHUNKS)
                                    for lhs, rhs in ((qtr_t, etr), (qtr_t, etl))
                                ] + [
                                    (qtl_t, etr, dc) for dc in range(ND_CHUNKS)
                                ]
                            else:
                                seq = [
                                    (lhs, rhs, dc)
                                    for lhs, rhs in pairs
                                    for dc in range(ND_CHUNKS)
                                ]
                            for i, (lhs, rhs, dc) in enumerate(seq):
                                nc.tensor.matmul(
                                    sc_ps[:, kh * 512 : (kh + 1) * 512],
                                    qsel(lhs, dc),
                                    rhs[:, dc, kh * 512 : (kh + 1) * 512],
                                    start=(i == 0),
                                    stop=(i == len(seq) - 1),
                                )
                    return sc_ps

                def emit_back(qb, sc_ps, b=b, e_r=e_r, e_rb=e_rb):
                    """Softmax block qb's scores, transpose W, bmm2, store."""
                    recip = stat_pool.tile([P, 1], F32, tag="recip", name="recip")
                    # tr_f32r: ACT writes the exp output pre-rounded to f32r
                    # (free on the output path) so the W transposes also run
                    # at 1.5 cyc/row
                    wdt = BF16 if w_bf16 else (F32R if tr_f32r else F32)
                    if w_split:
                        # two half-tiles: each half's W transposes start as
                        # soon as its own exp half is done
                        w_halves = [
                            w_pool.tile([P, T // 2], wdt, tag=f"w{h}", name=f"w{h}")
                            for h in range(2)
                        ]
                    else:
                        w_sb = w_pool.tile([P, T], wdt, name="w_sb")
                        w_halves = [w_sb[:, 0:512], w_sb[:, 512:1024]]
                    if no_softmax:
                        nc.scalar.copy(w_halves[0][:], sc_ps[:, 0:512])
                        nc.scalar.copy(w_halves[1][:], sc_ps[:, 512:1024])
                        nc.vector.memset(recip[:], 1.0)
                    else:
                        negmax = stat_pool.tile(
                            [P, 1], F32, tag="negmax", name="negmax"
                        )
                        if nm2:
                            # halve the row-max latency: the kh=0 PSUM bank is
                            # complete at bmm1's midpoint, so its reduce runs
                            # under the second half of bmm1; combine with min
                            nmh = [
                                stat_pool.tile(
                                    [P, 1], F32, tag=f"nmh{h}", name=f"nmh{h}"
                                )
                                for h in range(2)
                            ]
                            for h in range(2):
                                nc.vector.tensor_reduce(
                                    out=nmh[h][:],
                                    in_=sc_ps[:, h * 512 : (h + 1) * 512],
                                    op=mybir.AluOpType.max,
                                    axis=mybir.AxisListType.X,
                                    negate=True,
                                )
                            nc.vector.tensor_tensor(
                                negmax[:], nmh[0][:], nmh[1][:],
                                mybir.AluOpType.min,
                            )
                        else:
                            nc.vector.tensor_reduce(
                                out=negmax[:],
                                in_=sc_ps[:],
                                op=mybir.AluOpType.max,
                                axis=mybir.AxisListType.X,
                                negate=True,
                            )
                        if w_split:
                            ssums = [
                                stat_pool.tile(
                                    [P, 1], F32, tag=f"ssum{h}", name=f"ssum{h}"
                                )
                                for h in range(2)
                            ]
                            for h in range(2):
                                nc.scalar.activation(
                                    w_halves[h][:],
                                    sc_ps[:, h * 512 : (h + 1) * 512],
                                    mybir.ActivationFunctionType.Exp,
                                    bias=negmax[:],
                                    accum_out=ssums[h][:],
                                )
                            ssum = stat_pool.tile(
                                [P, 1], F32, tag="ssum", name="ssum"
                            )
                            nc.vector.tensor_tensor(
                                ssum[:], ssums[0][:], ssums[1][:],
                                mybir.AluOpType.add,
                            )
                        else:
                            ssum = stat_pool.tile(
                                [P, 1], F32, tag="ssum", name="ssum"
                            )
                            nc.scalar.activation(
                                w_sb[:],
                                sc_ps[:],
                                mybir.ActivationFunctionType.Exp,
                                bias=negmax[:],
                                accum_out=ssum[:],
                            )
                        nc.vector.reciprocal(recip[:], ssum[:])

                    wt_t = [
                        wt_pool.tile(
                            [P, NC_CHUNKS // ngr, P], wdt if w_bf16 else F32R,
                            tag=f"wt{h}", name=f"wt{h}",
                        )
                        for h in range(ngr)
                    ]
                    for g in range(NC_CHUNKS // 4):
                        wo = (g * 4) % nper
                        _transpose_block_group(
                            nc,
                            trans_psum,
                            ident_b[:]
                            if w_bf16
                            else (ident_r[:] if tr_f32r else ident[:]),
                            lambda j, g=g: w_halves[g][
                                :, (j * P) : (j + 1) * P
                            ],
                            wt_t[(g * 4) // nper][:, wo : wo + 4, :],
                            None,
                            copy_eng=cp_engs[(qb * 2 + g) % len(cp_engs)],
                            dtype=wdt,
                        )

                    # bmm2: ctx[q,d] = WT.T @ E
                    e2src = e_rb if w_bf16 else e_r
                    ctx_sb = ctx_pool.tile([P, D], F32, name="ctx_sb")
                    if bmm2_kc:
                        # kc-outer: both d-halves consume one weight load of
                        # wt[kc]; needs a 2-bank ctx PSUM tile
                        ctx_ps = ctx_psum.tile([P, T], F32, name="ctx_ps")
                        for kc in range(NC_CHUNKS):
                            for dh in range(2):
                                nc.tensor.matmul(
                                    ctx_ps[:, dh * 512 : (dh + 1) * 512],
                                    qsel(wt_t, kc),
                                    e2src[:, kc, dh * 512 : (dh + 1) * 512],
                                    start=(kc == 0),
                                    stop=(kc == NC_CHUNKS - 1),
                                )
                        for dh in range(2):
                            nc.vector.tensor_scalar_mul(
                                ctx_sb[:, dh * 512 : (dh + 1) * 512],
                                ctx_ps[:, dh * 512 : (dh + 1) * 512],
                                recip[:],
                            )
                    else:
                        # one PSUM bank per d-half; normalization copies
                        # split DVE/ACT so both halves drain in parallel
                        for dh in range(2):
                            ctx_ps = ctx_psum.tile([P, 512], F32, name="ctx_ps")
                            for kc in range(NC_CHUNKS):
                                nc.tensor.matmul(
                                    ctx_ps[:],
                                    qsel(wt_t, kc),
                                    e2src[:, kc, dh * 512 : (dh + 1) * 512],
                                    start=(kc == 0),
                                    stop=(kc == NC_CHUNKS - 1),
                                )
                            if ctx_act and dh == 1:
                                nc.scalar.activation(
                                    ctx_sb[:, dh * 512 : (dh + 1) * 512],
                                    ctx_ps[:],
                                    mybir.ActivationFunctionType.Copy,
                                    scale=recip[:],
                                )
                            else:
                                nc.vector.tensor_scalar_mul(
                                    ctx_sb[:, dh * 512 : (dh + 1) * 512],
                                    ctx_ps[:],
                                    recip[:],
                                )
                    nc.sync.dma_start(o_dram[b, qb * P : (qb + 1) * P, :], ctx_sb[:])

                # software pipeline: next block's bmm1 hides this block's softmax
                pend = emit_front(0)
                for qb in range(NC_CHUNKS):
                    nxt = emit_front(qb + 1) if qb + 1 < NC_CHUNKS else None
                    if (
                        e2
                        and qb == NC_CHUNKS - 1
                        and bi + 1 < len(batches)
                        and not e_hoist
                    ):
                        # prefetch next batch's E pipeline so the PE fills the
                        # last block's softmax/bmm2 shadow with E transposes
                        e_next = emit_e_setup(batches[bi + 1])
                    emit_back(qb, pend)
                    pend = nxt

    nc.compile()
    return nc


_NC_CACHE = None


def _get_nc():
    global _NC_CACHE
    if _NC_CACHE is None:
        _NC_CACHE = build_nc()
    return _NC_CACHE


def kernel(decoder_hidden: np.ndarray, encoder_outputs: np.ndarray) -> np.ndarray:
    import os

    # The axon client here has no NTFF profiling hook; make sure a stray
    # BASS_TRACE in the environment can't push run_bass_kernel_spmd onto
    # the tracing path.
    os.environ["BASS_NEVER_TRACE"] = "1"
    from concourse import bass_utils

    dh = np.ascontiguousarray(np.asarray(decoder_hidden, dtype=np.float32))
    eo = np.ascontiguousarray(np.asarray(encoder_outputs, dtype=np.float32))
    assert dh.shape == (16, T, D) and eo.shape == (16, T, D)

    nc = _get_nc()
    in_maps = [
        {
            "q": dh[i * B_PER_CORE : (i + 1) * B_PER_CORE],
            "e": eo[i * B_PER_CORE : (i + 1) * B_PER_CORE],
        }
        for i in range(8)
    ]
    res = bass_utils.run_bass_kernel_spmd(nc, in_maps, core_ids=list(range(8)))
    return np.concatenate([r["o"] for r in res.results], axis=0)

